# revision 21
# baseline (speedup 1.0000x reference)
"""Trainium2 Bass kernel for the gnn_message_passing encoder problem.

kernel(**inputs) takes the FULL inputs and returns the FULL [B, P, R+1] output.

Sharding: 8 cores = 2 batches x 4 object-groups.  Each core scores ~62
(trigger, object) pairs of one document.  The host shards inputs per core:
only the attention rows touched by the core's entity spans are shipped
(laid out dense as [head, entity, w, L]), plus the full sequence_output of
its batch (needed by the context matmul), the span token rows, small
one-hot selection matrices (with the 1/W span-mean folded in), and the
transposed relation/nota codebooks.  All arithmetic (span means, pair
products, head sums, normalization, context matmul, scoring, NOTA max)
runs on device.
"""

import os
import sys

import numpy as np

for _p in ("/opt/trn_rl_repo", os.path.expanduser("~/.axon_site/_ro/trn_rl_repo")):
    if os.path.isdir(_p) and _p not in sys.path:
        sys.path.insert(0, _p)

import concourse.bass as bass
import concourse.mybir as mybir
import concourse.tile as tile
from concourse import bacc
from concourse.bass_utils import run_bass_kernel_spmd

# Problem dimensions (hardcoded per the harness contract).
B, L, D, H = 2, 2048, 768, 12
E, T, W = 32, 8, 4
R, NN = 57, 20
RN = R + NN            # 77 stacked codebook rows
F = 3 * D              # 2304 concat feature dim
NE = 16                # entities per core (8 triggers + 8 objects)
NEW = NE * W           # 64 gathered rows per head
NP = 64                # pair slots per core (group 0 pads 56 -> 64)
LQ = 512               # L is processed in 4 slices of 512
NCORES = 8

# Static pair list in the reference's order (s-major).
ALL_PAIRS = [(s, o) for s in range(T) for o in range(E) if s != o]
GROUP_IDX = [[i for i, (_, o) in enumerate(ALL_PAIRS) if o // 8 == g] for g in range(4)]
GROUP_ENTS = [
    list(range(16)),
    list(range(16)),
    list(range(8)) + list(range(16, 24)),
    list(range(8)) + list(range(24, 32)),
]

F32 = mybir.dt.float32
BF16 = mybir.dt.bfloat16
import ml_dtypes
NP_BF16 = ml_dtypes.bfloat16

LAST_RESULTS = None  # BassKernelResults of the most recent kernel() call

FOLD2 = np.ascontiguousarray(
    np.concatenate([np.eye(NP), np.eye(NP)], axis=0).astype(np.float32)
)


def _sel_matrices(g):
    """[NEW, NP] one-hot (x 0.25) selectors for the s and o side of each pair."""
    idxs = GROUP_IDX[g]
    ents = GROUP_ENTS[g]
    local = {e: i for i, e in enumerate(ents)}
    sel_s = np.zeros((NEW, NP), np.float32)
    sel_o = np.zeros((NEW, NP), np.float32)
    for j in range(NP):
        s, o = ALL_PAIRS[idxs[j % len(idxs)]]  # pad group 0 by repeating pair 0
        for w in range(W):
            sel_s[local[s] * W + w, j] = 0.25
            sel_o[local[o] * W + w, j] = 0.25
    return sel_s, sel_o


def _build_program():
    nc = bacc.Bacc("TRN2")

    att_g = nc.dram_tensor("att_g", [H * NEW, L], BF16, kind="ExternalInput")
    seq = nc.dram_tensor("seq", [L, D], BF16, kind="ExternalInput")
    spans = nc.dram_tensor("spans", [NEW, D], BF16, kind="ExternalInput")
    sel_s = nc.dram_tensor("sel_s", [2 * NEW, NP], BF16, kind="ExternalInput")
    sel_o = nc.dram_tensor("sel_o", [2 * NEW, NP], BF16, kind="ExternalInput")
    rel_t = nc.dram_tensor("rel_t", [F, RN], BF16, kind="ExternalInput")
    fold2 = nc.dram_tensor("fold2", [128, NP], F32, kind="ExternalInput")
    out = nc.dram_tensor("out", [NP, R + 1], F32, kind="ExternalOutput")

    with tile.TileContext(nc) as tc:
        with tc.tile_pool(name="consts", bufs=1) as consts:
            # Small inputs first so the early stages can start immediately.
            sels_sb = consts.tile([2 * NEW, NP], BF16)
            nc.sync.dma_start(out=sels_sb, in_=sel_s[:, :])
            selo_sb = consts.tile([2 * NEW, NP], BF16)
            nc.sync.dma_start(out=selo_sb, in_=sel_o[:, :])
            # Attention rows split by L-quarter so stage A pipelines with DMA.
            g_sb = consts.tile([128, 6, L], BF16)      # [h,e,w] rows: 2 heads/tile
            g_view = att_g.rearrange("(t p) l -> p t l", p=128)
            for lq in range(2):
                nc.sync.dma_start(out=g_sb[:, :, lq * LQ:(lq + 1) * LQ],
                                  in_=g_view[:, :, lq * LQ:(lq + 1) * LQ])
            spans_sb = consts.tile([NEW, D], BF16)
            nc.sync.dma_start(out=spans_sb, in_=spans[:, :])
            fold2_sb = consts.tile([128, NP], F32)
            nc.sync.dma_start(out=fold2_sb, in_=fold2[:, :])
            for lq in range(2, 4):
                nc.sync.dma_start(out=g_sb[:, :, lq * LQ:(lq + 1) * LQ],
                                  in_=g_view[:, :, lq * LQ:(lq + 1) * LQ])
            seq_sb = consts.tile([128, 16, D], BF16)   # 16 L-chunks of [128, D]
            nc.sync.dma_start(out=seq_sb, in_=seq.rearrange("(c p) d -> p c d", p=128))
            rel_sb = consts.tile([128, 18, RN], BF16)  # 18 K-chunks of [128, RN]
            nc.sync.dma_start(out=rel_sb, in_=rel_t.rearrange("(c p) n -> p c n", p=128))
            id_sb = consts.tile([128, 128], F32)
            nc.gpsimd.memset(id_sb, 0.0)
            nc.gpsimd.affine_select(
                out=id_sb, in_=id_sb,
                compare_op=mybir.AluOpType.not_equal, fill=1.0, base=0,
                pattern=[[-1, 128]], channel_multiplier=1,
            )

            q_sb = consts.tile([NP, L], F32)
            aT_sb = consts.tile([128, 16, NP], BF16)
            embsT = consts.tile([128, 18, NP], BF16)
            fin = consts.tile([NP, R + 1], F32)

            # PSUM budget: psA(2x2=4) + psQ(2, shared tag) + psC(2) = 8 banks.
            with tc.tile_pool(name="psA", bufs=2, space="PSUM") as psA, \
                 tc.tile_pool(name="psQ", bufs=2, space="PSUM") as psQ, \
                 tc.tile_pool(name="psC", bufs=1, space="PSUM") as psC, \
                 tc.tile_pool(name="prod", bufs=3) as prod:

                # Stage E (emitted first so it fills early PE gaps):
                # entity embeddings, span mean folded into the selectors.
                for dt in range(6):
                    ee_ps = psQ.tile([128, 2 * NP], F32, tag="qe")
                    sp_sl = spans_sb[:, dt * 128:(dt + 1) * 128]
                    nc.tensor.matmul(out=ee_ps[:, 0:NP], lhsT=sp_sl, rhs=sels_sb[0:64, :])
                    nc.tensor.matmul(out=ee_ps[:, NP:2 * NP], lhsT=sp_sl, rhs=selo_sb[0:64, :])
                    nc.scalar.copy(embsT[:, dt, :], ee_ps[:, 0:NP])
                    nc.scalar.copy(embsT[:, 6 + dt, :], ee_ps[:, NP:2 * NP])

                # Stage A: per-pair attention rows via one-hot matmuls
                # (2 heads stacked per PSUM tile), pair products + head sums.
                # The context matmul (stage D) consumes unnormalized qT chunks
                # as they appear, overlapping with stage A; the 1/rowsum(q)
                # scale is folded into the PSUM->embsT copy at the end.
                c_ps0 = psC.tile([NP, 384], F32, tag="c0")
                c_ps1 = psC.tile([NP, 384], F32, tag="c1")
                qp4 = consts.tile([NP, 4], F32)
                for lq in range(4):
                    pm = prod.tile([128, 6, LQ], F32, tag="prods")
                    for hp in range(6):
                        a_s = psA.tile([128, LQ], F32, tag="as")
                        a_o = psA.tile([128, LQ], F32, tag="ao")
                        lo = g_sb[0:64, hp, lq * LQ:(lq + 1) * LQ]
                        hi = g_sb[64:128, hp, lq * LQ:(lq + 1) * LQ]
                        nc.tensor.matmul(out=a_s[0:64, :], lhsT=sels_sb[0:64, :], rhs=lo)
                        nc.tensor.matmul(out=a_s[64:128, :], lhsT=sels_sb[64:128, :], rhs=hi)
                        nc.tensor.matmul(out=a_o[0:64, :], lhsT=selo_sb[0:64, :], rhs=lo)
                        nc.tensor.matmul(out=a_o[64:128, :], lhsT=selo_sb[64:128, :], rhs=hi)
                        # 2-input DVE ops may read at most one PSUM operand:
                        # stage the S side through SBUF on the scalar engine
                        as_sb = prod.tile([128, LQ], F32, tag="as_sb")
                        nc.scalar.copy(as_sb, a_s)
                        nc.vector.tensor_mul(pm[:, hp, :], as_sb, a_o)
                    # head-sum tree, merged ops
                    nc.vector.tensor_add(pm[:, 0:3, :], pm[:, 0:3, :], pm[:, 3:6, :])
                    nc.vector.tensor_add(pm[:, 0, :], pm[:, 0, :], pm[:, 1, :])
                    nc.vector.tensor_add(pm[:, 0, :], pm[:, 0, :], pm[:, 2, :])
                    # fold the two stacked head-halves across partitions
                    qt = psQ.tile([NP, LQ], F32, tag="qe")
                    nc.tensor.matmul(out=qt, lhsT=fold2_sb, rhs=pm[:, 0, :])
                    nc.vector.reduce_sum(qp4[:, lq:lq + 1], qt, axis=mybir.AxisListType.X)
                    nc.scalar.copy(q_sb[:, lq * LQ:(lq + 1) * LQ], qt)
                    for k in range(4):
                        ch = lq * 4 + k
                        qT_ps = psQ.tile([128, NP], F32, tag="qe")
                        nc.tensor.transpose(
                            qT_ps, q_sb[:, ch * 128:(ch + 1) * 128], id_sb[0:NP, 0:NP]
                        )
                        nc.vector.tensor_copy(aT_sb[:, ch, :], qT_ps)
                        # stage D overlapped: c_raw[p, d] += qT[ch].T @ seq[ch]
                        nc.tensor.matmul(
                            out=c_ps0, lhsT=aT_sb[:, ch, :],
                            rhs=seq_sb[:, ch, 0:384],
                            start=(ch == 0), stop=(ch == 15),
                        )
                        nc.tensor.matmul(
                            out=c_ps1, lhsT=aT_sb[:, ch, :],
                            rhs=seq_sb[:, ch, 384:768],
                            start=(ch == 0), stop=(ch == 15),
                        )

                # Stage B: normalize c by 1/rowsum(q) (per-partition scalar)
                qsum = consts.tile([NP, 1], F32)
                nc.vector.reduce_sum(qsum, qp4, axis=mybir.AxisListType.X)
                rq = consts.tile([NP, 1], F32)
                nc.vector.reciprocal(rq, qsum)
                c_sb = consts.tile([NP, D], F32)
                nc.vector.tensor_scalar_mul(c_sb[:, 0:384], c_ps0, rq)
                nc.vector.tensor_scalar_mul(c_sb[:, 384:768], c_ps1, rq)
                # transpose c into embsT layout [d, p] (bf16)
                for dt in range(6):
                    cT_ps = psQ.tile([128, NP], F32, tag="qe")
                    nc.tensor.transpose(
                        cT_ps, c_sb[:, dt * 128:(dt + 1) * 128], id_sb[0:NP, 0:NP]
                    )
                    nc.vector.tensor_copy(embsT[:, 12 + dt, :], cT_ps)

            with tc.tile_pool(name="psF", bufs=1, space="PSUM") as psF:
                # Stage F: scores = [rel; nota] @ embs, then transpose + NOTA max
                sc_ps = psF.tile([RN, NP], F32, tag="sc")
                for kc in range(18):
                    nc.tensor.matmul(
                        out=sc_ps,
                        lhsT=rel_sb[:, kc, :],
                        rhs=embsT[:, kc, :],
                        start=(kc == 0),
                        stop=(kc == 17),
                    )
                sc_sb = consts.tile([RN, NP], F32)
                nc.vector.tensor_copy(sc_sb, sc_ps)
                scT_ps = psF.tile([NP, RN], F32, tag="scT")
                nc.tensor.transpose(scT_ps, sc_sb, id_sb[0:RN, 0:RN])
                nota = consts.tile([NP, 1], F32)
                nc.vector.reduce_max(nota, scT_ps[:, R:RN], axis=mybir.AxisListType.X)
                nc.vector.tensor_copy(fin[:, 1:R + 1], scT_ps[:, 0:R])
                nc.vector.tensor_copy(fin[:, 0:1], nota)

            nc.sync.dma_start(out=out[:, :], in_=fin)

    return nc


def kernel(sequence_output, attention, relation_embeddings, nota_embeddings,
           span_starts):
    global LAST_RESULTS
    sequence_output = np.asarray(sequence_output, np.float32)
    attention = np.asarray(attention, np.float32)
    span_starts = np.asarray(span_starts)
    rel_t = np.ascontiguousarray(
        np.concatenate(
            [np.asarray(relation_embeddings, np.float32),
             np.asarray(nota_embeddings, np.float32)], axis=0
        ).T
    )

    in_maps = []
    for c in range(NCORES):
        b, g = divmod(c, 4)
        ents = GROUP_ENTS[g]
        rows = np.concatenate(
            [np.arange(span_starts[b, e], span_starts[b, e] + W) for e in ents]
        )
        att_rows = attention[b][:, rows, :].reshape(H * NEW, L)
        sel_s, sel_o = _sel_matrices(g)
        in_maps.append({
            "att_g": np.ascontiguousarray(att_rows.astype(NP_BF16)),
            "seq": np.ascontiguousarray(sequence_output[b].astype(NP_BF16)),
            "spans": np.ascontiguousarray(sequence_output[b][rows].astype(NP_BF16)),
            "sel_s": np.ascontiguousarray(np.concatenate([sel_s, sel_s], 0).astype(NP_BF16)),
            "sel_o": np.ascontiguousarray(np.concatenate([sel_o, sel_o], 0).astype(NP_BF16)),
            "rel_t": rel_t.astype(NP_BF16),
            "fold2": FOLD2,
        })

    nc = _build_program()
    nc.finalize()  # Bacc legalization (wait splitting, reg alloc)
    LAST_RESULTS = run_bass_kernel_spmd(nc, in_maps, core_ids=list(range(NCORES)))

    out = np.zeros((B, len(ALL_PAIRS), R + 1), np.float32)
    for c in range(NCORES):
        b, g = divmod(c, 4)
        idxs = GROUP_IDX[g]
        out[b, idxs, :] = LAST_RESULTS.results[c]["out"][: len(idxs)]
    return out


# revision 23
# speedup vs baseline: 1.0817x; 1.0817x over previous
"""Trainium2 Bass kernel for the gnn_message_passing encoder problem.

kernel(**inputs) takes the FULL inputs and returns the FULL [B, P, R+1] output.

Sharding: 8 cores = 2 batches x 4 object-groups.  Each core scores ~62
(trigger, object) pairs of one document.  The host shards inputs per core:
only the attention rows touched by the core's entity spans are shipped
(laid out dense as [head, entity, w, L]), plus the full sequence_output of
its batch (needed by the context matmul), the span token rows, small
one-hot selection matrices (with the 1/W span-mean folded in), and the
transposed relation/nota codebooks.  All arithmetic (span means, pair
products, head sums, normalization, context matmul, scoring, NOTA max)
runs on device.
"""

import os
import sys

import numpy as np

for _p in ("/opt/trn_rl_repo", os.path.expanduser("~/.axon_site/_ro/trn_rl_repo")):
    if os.path.isdir(_p) and _p not in sys.path:
        sys.path.insert(0, _p)

import concourse.bass as bass
import concourse.mybir as mybir
import concourse.tile as tile
from concourse import bacc
from concourse.bass_utils import run_bass_kernel_spmd

# Problem dimensions (hardcoded per the harness contract).
B, L, D, H = 2, 2048, 768, 12
E, T, W = 32, 8, 4
R, NN = 57, 20
RN = R + NN            # 77 stacked codebook rows
F = 3 * D              # 2304 concat feature dim
NE = 16                # entities per core (8 triggers + 8 objects)
NEW = NE * W           # 64 gathered rows per head
NP = 64                # pair slots per core (group 0 pads 56 -> 64)
LQ = 512               # L is processed in 4 slices of 512
NCORES = 8

# Static pair list in the reference's order (s-major).
ALL_PAIRS = [(s, o) for s in range(T) for o in range(E) if s != o]
GROUP_IDX = [[i for i, (_, o) in enumerate(ALL_PAIRS) if o // 8 == g] for g in range(4)]
GROUP_ENTS = [
    list(range(16)),
    list(range(16)),
    list(range(8)) + list(range(16, 24)),
    list(range(8)) + list(range(24, 32)),
]

F32 = mybir.dt.float32
BF16 = mybir.dt.bfloat16
import ml_dtypes
NP_BF16 = ml_dtypes.bfloat16

LAST_RESULTS = None  # BassKernelResults of the most recent kernel() call

FOLD2 = np.ascontiguousarray(
    np.concatenate([np.eye(NP), np.eye(NP)], axis=0).astype(np.float32)
)


def _sel_matrices(g):
    """[NEW, NP] one-hot (x 0.25) selectors for the s and o side of each pair."""
    idxs = GROUP_IDX[g]
    ents = GROUP_ENTS[g]
    local = {e: i for i, e in enumerate(ents)}
    sel_s = np.zeros((NEW, NP), np.float32)
    sel_o = np.zeros((NEW, NP), np.float32)
    for j in range(NP):
        s, o = ALL_PAIRS[idxs[j % len(idxs)]]  # pad group 0 by repeating pair 0
        for w in range(W):
            sel_s[local[s] * W + w, j] = 0.25
            sel_o[local[o] * W + w, j] = 0.25
    return sel_s, sel_o


def _build_program():
    nc = bacc.Bacc("TRN2")

    att_g = nc.dram_tensor("att_g", [H * NEW, L], BF16, kind="ExternalInput")
    seq = nc.dram_tensor("seq", [L, D], BF16, kind="ExternalInput")
    spans = nc.dram_tensor("spans", [NEW, D], BF16, kind="ExternalInput")
    sel_s = nc.dram_tensor("sel_s", [2 * NEW, NP], BF16, kind="ExternalInput")
    sel_o = nc.dram_tensor("sel_o", [2 * NEW, NP], BF16, kind="ExternalInput")
    rel_t = nc.dram_tensor("rel_t", [F, RN], BF16, kind="ExternalInput")
    fold2 = nc.dram_tensor("fold2", [128, NP], F32, kind="ExternalInput")
    out = nc.dram_tensor("out", [NP, R + 1], F32, kind="ExternalOutput")

    with tile.TileContext(nc) as tc:
        with tc.tile_pool(name="consts", bufs=1) as consts:
            # Small inputs first so the early stages can start immediately.
            sels_sb = consts.tile([2 * NEW, NP], BF16)
            nc.sync.dma_start(out=sels_sb, in_=sel_s[:, :])
            selo_sb = consts.tile([2 * NEW, NP], BF16)
            nc.sync.dma_start(out=selo_sb, in_=sel_o[:, :])
            spans_sb = consts.tile([NEW, D], BF16)
            nc.sync.dma_start(out=spans_sb, in_=spans[:, :])
            fold2_sb = consts.tile([128, NP], F32)
            nc.sync.dma_start(out=fold2_sb, in_=fold2[:, :])
            # Attention rows split by L-quarter so stage A pipelines with DMA.
            g_sb = consts.tile([128, 6, L], BF16)      # [h,e,w] rows: 2 heads/tile
            g_view = att_g.rearrange("(t p) l -> p t l", p=128)
            for lq in range(4):
                nc.sync.dma_start(out=g_sb[:, :, lq * LQ:(lq + 1) * LQ],
                                  in_=g_view[:, :, lq * LQ:(lq + 1) * LQ])
            seq_sb = consts.tile([128, 16, D], BF16)   # 16 L-chunks of [128, D]
            nc.sync.dma_start(out=seq_sb, in_=seq.rearrange("(c p) d -> p c d", p=128))
            rel_sb = consts.tile([128, 18, RN], BF16)  # 18 K-chunks of [128, RN]
            nc.sync.dma_start(out=rel_sb, in_=rel_t.rearrange("(c p) n -> p c n", p=128))
            id_sb = consts.tile([128, 128], F32)
            nc.gpsimd.memset(id_sb, 0.0)
            nc.gpsimd.affine_select(
                out=id_sb, in_=id_sb,
                compare_op=mybir.AluOpType.not_equal, fill=1.0, base=0,
                pattern=[[-1, 128]], channel_multiplier=1,
            )

            q_sb = consts.tile([NP, L], F32)
            aT_sb = consts.tile([128, 16, NP], BF16)
            embsT = consts.tile([128, 18, NP], BF16)
            fin = consts.tile([NP, R + 1], F32)

            # PSUM budget: psA(3x2=6) + psQ(2, shared tag) = 8 banks.
            with tc.tile_pool(name="psA", bufs=3, space="PSUM") as psA, \
                 tc.tile_pool(name="psQ", bufs=2, space="PSUM") as psQ, \
                 tc.tile_pool(name="prod", bufs=3) as prod:

                # Stage E (emitted first so it fills early PE gaps):
                # entity embeddings, span mean folded into the selectors.
                for dt in range(6):
                    ee_ps = psQ.tile([128, 2 * NP], F32, tag="qe")
                    sp_sl = spans_sb[:, dt * 128:(dt + 1) * 128]
                    nc.tensor.matmul(out=ee_ps[:, 0:NP], lhsT=sp_sl, rhs=sels_sb[0:64, :])
                    nc.tensor.matmul(out=ee_ps[:, NP:2 * NP], lhsT=sp_sl, rhs=selo_sb[0:64, :])
                    nc.scalar.copy(embsT[:, dt, :], ee_ps[:, 0:NP])
                    nc.scalar.copy(embsT[:, 6 + dt, :], ee_ps[:, NP:2 * NP])

                # Stage A: per-pair attention rows via one-hot matmuls
                # (2 heads stacked per PSUM tile), pair products + head sums.
                qp4 = consts.tile([NP, 4], F32)
                for lq in range(4):
                    pm = prod.tile([128, 6, LQ], F32, tag="prods")
                    for hp in range(6):
                        a_s = psA.tile([128, LQ], F32, tag="as")
                        a_o = psA.tile([128, LQ], F32, tag="ao")
                        lo = g_sb[0:64, hp, lq * LQ:(lq + 1) * LQ]
                        hi = g_sb[64:128, hp, lq * LQ:(lq + 1) * LQ]
                        nc.tensor.matmul(out=a_s[0:64, :], lhsT=sels_sb[0:64, :], rhs=lo)
                        nc.tensor.matmul(out=a_s[64:128, :], lhsT=sels_sb[64:128, :], rhs=hi)
                        nc.tensor.matmul(out=a_o[0:64, :], lhsT=selo_sb[0:64, :], rhs=lo)
                        nc.tensor.matmul(out=a_o[64:128, :], lhsT=selo_sb[64:128, :], rhs=hi)
                        # 2-input DVE ops may read at most one PSUM operand:
                        # stage the S side through SBUF on the scalar engine
                        as_sb = prod.tile([128, LQ], F32, tag="as_sb")
                        nc.scalar.copy(as_sb, a_s)
                        nc.vector.tensor_mul(pm[:, hp, :], as_sb, a_o)
                    # head-sum tree, merged ops
                    nc.vector.tensor_add(pm[:, 0:3, :], pm[:, 0:3, :], pm[:, 3:6, :])
                    nc.vector.tensor_add(pm[:, 0, :], pm[:, 0, :], pm[:, 1, :])
                    nc.vector.tensor_add(pm[:, 0, :], pm[:, 0, :], pm[:, 2, :])
                    # fold the two stacked head-halves across partitions
                    qt = psQ.tile([NP, LQ], F32, tag="qe")
                    nc.tensor.matmul(out=qt, lhsT=fold2_sb, rhs=pm[:, 0, :])
                    nc.vector.reduce_sum(qp4[:, lq:lq + 1], qt, axis=mybir.AxisListType.X)
                    nc.scalar.copy(q_sb[:, lq * LQ:(lq + 1) * LQ], qt)
                    for k in range(4):
                        ch = lq * 4 + k
                        qT_ps = psQ.tile([128, NP], F32, tag="qe")
                        nc.tensor.transpose(
                            qT_ps, q_sb[:, ch * 128:(ch + 1) * 128], id_sb[0:NP, 0:NP]
                        )
                        nc.vector.tensor_copy(aT_sb[:, ch, :], qT_ps)

                # Stage B: 1/rowsum(q)
                qsum = consts.tile([NP, 1], F32)
                nc.vector.reduce_sum(qsum, qp4, axis=mybir.AxisListType.X)
                rq = consts.tile([NP, 1], F32)
                nc.vector.reciprocal(rq, qsum)

            with tc.tile_pool(name="psC", bufs=1, space="PSUM") as psC, \
                 tc.tile_pool(name="psF", bufs=1, space="PSUM") as psF:
                # Stage D: c_raw[p, d] = qT.T @ seq, accumulated over L-chunks
                c_ps0 = psC.tile([NP, 384], F32, tag="c0")
                c_ps1 = psC.tile([NP, 384], F32, tag="c1")
                for ch in range(16):
                    nc.tensor.matmul(out=c_ps0, lhsT=aT_sb[:, ch, :],
                                     rhs=seq_sb[:, ch, 0:384],
                                     start=(ch == 0), stop=(ch == 15))
                    nc.tensor.matmul(out=c_ps1, lhsT=aT_sb[:, ch, :],
                                     rhs=seq_sb[:, ch, 384:768],
                                     start=(ch == 0), stop=(ch == 15))
                # normalize by 1/rowsum(q) (per-partition scalar), transpose
                # into embsT layout [d, p] (bf16)
                c_sb = consts.tile([NP, D], F32)
                nc.vector.tensor_scalar_mul(c_sb[:, 0:384], c_ps0, rq)
                nc.vector.tensor_scalar_mul(c_sb[:, 384:768], c_ps1, rq)
                for dt in range(6):
                    cT_ps = psC.tile([128, NP], F32, tag="cT", bufs=2)
                    nc.tensor.transpose(
                        cT_ps, c_sb[:, dt * 128:(dt + 1) * 128], id_sb[0:NP, 0:NP]
                    )
                    nc.vector.tensor_copy(embsT[:, 12 + dt, :], cT_ps)
                # Stage F: scores = [rel; nota] @ embs, then transpose + NOTA max
                sc_ps = psF.tile([RN, NP], F32, tag="sc")
                for kc in range(18):
                    nc.tensor.matmul(
                        out=sc_ps,
                        lhsT=rel_sb[:, kc, :],
                        rhs=embsT[:, kc, :],
                        start=(kc == 0),
                        stop=(kc == 17),
                    )
                sc_sb = consts.tile([RN, NP], F32)
                nc.vector.tensor_copy(sc_sb, sc_ps)
                scT_ps = psF.tile([NP, RN], F32, tag="scT")
                nc.tensor.transpose(scT_ps, sc_sb, id_sb[0:RN, 0:RN])
                nota = consts.tile([NP, 1], F32)
                nc.vector.reduce_max(nota, scT_ps[:, R:RN], axis=mybir.AxisListType.X)
                nc.vector.tensor_copy(fin[:, 1:R + 1], scT_ps[:, 0:R])
                nc.vector.tensor_copy(fin[:, 0:1], nota)

            nc.sync.dma_start(out=out[:, :], in_=fin)

    return nc


def kernel(sequence_output, attention, relation_embeddings, nota_embeddings,
           span_starts):
    global LAST_RESULTS
    sequence_output = np.asarray(sequence_output, np.float32)
    attention = np.asarray(attention, np.float32)
    span_starts = np.asarray(span_starts)
    rel_t = np.ascontiguousarray(
        np.concatenate(
            [np.asarray(relation_embeddings, np.float32),
             np.asarray(nota_embeddings, np.float32)], axis=0
        ).T
    )

    in_maps = []
    for c in range(NCORES):
        b, g = divmod(c, 4)
        ents = GROUP_ENTS[g]
        rows = np.concatenate(
            [np.arange(span_starts[b, e], span_starts[b, e] + W) for e in ents]
        )
        att_rows = attention[b][:, rows, :].reshape(H * NEW, L)
        sel_s, sel_o = _sel_matrices(g)
        in_maps.append({
            "att_g": np.ascontiguousarray(att_rows.astype(NP_BF16)),
            "seq": np.ascontiguousarray(sequence_output[b].astype(NP_BF16)),
            "spans": np.ascontiguousarray(sequence_output[b][rows].astype(NP_BF16)),
            "sel_s": np.ascontiguousarray(np.concatenate([sel_s, sel_s], 0).astype(NP_BF16)),
            "sel_o": np.ascontiguousarray(np.concatenate([sel_o, sel_o], 0).astype(NP_BF16)),
            "rel_t": rel_t.astype(NP_BF16),
            "fold2": FOLD2,
        })

    nc = _build_program()
    nc.finalize()  # Bacc legalization (wait splitting, reg alloc)
    LAST_RESULTS = run_bass_kernel_spmd(nc, in_maps, core_ids=list(range(NCORES)))

    out = np.zeros((B, len(ALL_PAIRS), R + 1), np.float32)
    for c in range(NCORES):
        b, g = divmod(c, 4)
        idxs = GROUP_IDX[g]
        out[b, idxs, :] = LAST_RESULTS.results[c]["out"][: len(idxs)]
    return out


# revision 24
# speedup vs baseline: 1.2389x; 1.1453x over previous
"""Trainium2 Bass kernel for the gnn_message_passing encoder problem.

kernel(**inputs) takes the FULL inputs and returns the FULL [B, P, R+1] output.

Sharding: 8 cores = 2 batches x 4 object-groups.  Each core scores ~62
(trigger, object) pairs of one document.  The host shards inputs per core:
only the attention rows touched by the core's entity spans are shipped
(laid out dense as [head, entity, w, L]), plus the full sequence_output of
its batch (needed by the context matmul), the span token rows, small
one-hot selection matrices (with the 1/W span-mean folded in), and the
transposed relation/nota codebooks.  All arithmetic (span means, pair
products, head sums, normalization, context matmul, scoring, NOTA max)
runs on device.
"""

import os
import sys

import numpy as np

for _p in ("/opt/trn_rl_repo", os.path.expanduser("~/.axon_site/_ro/trn_rl_repo")):
    if os.path.isdir(_p) and _p not in sys.path:
        sys.path.insert(0, _p)

import concourse.bass as bass
import concourse.mybir as mybir
import concourse.tile as tile
from concourse import bacc
from concourse.bass_utils import run_bass_kernel_spmd

# Problem dimensions (hardcoded per the harness contract).
B, L, D, H = 2, 2048, 768, 12
E, T, W = 32, 8, 4
R, NN = 57, 20
RN = R + NN            # 77 stacked codebook rows
F = 3 * D              # 2304 concat feature dim
NE = 16                # entities per core (8 triggers + 8 objects)
NEW = NE * W           # 64 gathered rows per head
NP = 64                # pair slots per core (group 0 pads 56 -> 64)
LQ = 512               # L is processed in 4 slices of 512
NCORES = 8

# Static pair list in the reference's order (s-major).
ALL_PAIRS = [(s, o) for s in range(T) for o in range(E) if s != o]
GROUP_IDX = [[i for i, (_, o) in enumerate(ALL_PAIRS) if o // 8 == g] for g in range(4)]
GROUP_ENTS = [
    list(range(16)),
    list(range(16)),
    list(range(8)) + list(range(16, 24)),
    list(range(8)) + list(range(24, 32)),
]

F32 = mybir.dt.float32
BF16 = mybir.dt.bfloat16
import ml_dtypes
NP_BF16 = ml_dtypes.bfloat16

LAST_RESULTS = None  # BassKernelResults of the most recent kernel() call

FOLD2 = np.ascontiguousarray(
    np.concatenate([np.eye(NP), np.eye(NP)], axis=0).astype(np.float32)
)


def _sel_matrices(g):
    """[NEW, NP] one-hot (x 0.25) selectors for the s and o side of each pair."""
    idxs = GROUP_IDX[g]
    ents = GROUP_ENTS[g]
    local = {e: i for i, e in enumerate(ents)}
    sel_s = np.zeros((NEW, NP), np.float32)
    sel_o = np.zeros((NEW, NP), np.float32)
    for j in range(NP):
        s, o = ALL_PAIRS[idxs[j % len(idxs)]]  # pad group 0 by repeating pair 0
        for w in range(W):
            sel_s[local[s] * W + w, j] = 0.25
            sel_o[local[o] * W + w, j] = 0.25
    return sel_s, sel_o


def _build_program():
    nc = bacc.Bacc("TRN2")

    att_g = nc.dram_tensor("att_g", [H * NEW, L], BF16, kind="ExternalInput")
    seq = nc.dram_tensor("seq", [L, D], BF16, kind="ExternalInput")
    spans = nc.dram_tensor("spans", [NEW, D], BF16, kind="ExternalInput")
    sel_s = nc.dram_tensor("sel_s", [2 * NEW, NP], BF16, kind="ExternalInput")
    sel_o = nc.dram_tensor("sel_o", [2 * NEW, NP], BF16, kind="ExternalInput")
    rel_t = nc.dram_tensor("rel_t", [F, RN], BF16, kind="ExternalInput")
    fold2 = nc.dram_tensor("fold2", [128, NP], BF16, kind="ExternalInput")
    out = nc.dram_tensor("out", [NP, R + 1], F32, kind="ExternalOutput")

    with tile.TileContext(nc) as tc:
        with tc.tile_pool(name="consts", bufs=1) as consts:
            # Small inputs first so the early stages can start immediately.
            sels_sb = consts.tile([2 * NEW, NP], BF16)
            nc.sync.dma_start(out=sels_sb, in_=sel_s[:, :])
            selo_sb = consts.tile([2 * NEW, NP], BF16)
            nc.sync.dma_start(out=selo_sb, in_=sel_o[:, :])
            spans_sb = consts.tile([NEW, D], BF16)
            nc.sync.dma_start(out=spans_sb, in_=spans[:, :])
            fold2_sb = consts.tile([128, NP], BF16)
            nc.sync.dma_start(out=fold2_sb, in_=fold2[:, :])
            # Attention rows split by L-quarter so stage A pipelines with DMA.
            g_sb = consts.tile([128, 6, L], BF16)      # [h,e,w] rows: 2 heads/tile
            g_view = att_g.rearrange("(t p) l -> p t l", p=128)
            for lq in range(4):
                nc.sync.dma_start(out=g_sb[:, :, lq * LQ:(lq + 1) * LQ],
                                  in_=g_view[:, :, lq * LQ:(lq + 1) * LQ])
            seq_sb = consts.tile([128, 16, D], BF16)   # 16 L-chunks of [128, D]
            nc.sync.dma_start(out=seq_sb, in_=seq.rearrange("(c p) d -> p c d", p=128))
            rel_sb = consts.tile([128, 18, RN], BF16)  # 18 K-chunks of [128, RN]
            nc.sync.dma_start(out=rel_sb, in_=rel_t.rearrange("(c p) n -> p c n", p=128))
            id_sb = consts.tile([128, 128], F32)
            nc.gpsimd.memset(id_sb, 0.0)
            nc.gpsimd.affine_select(
                out=id_sb, in_=id_sb,
                compare_op=mybir.AluOpType.not_equal, fill=1.0, base=0,
                pattern=[[-1, 128]], channel_multiplier=1,
            )

            q_sb = consts.tile([NP, L], F32)
            aT_sb = consts.tile([128, 16, NP], BF16)
            embsT = consts.tile([128, 18, NP], BF16)
            fin = consts.tile([NP, R + 1], F32)

            # PSUM budget: psA(3x2=6) + psQ(2, shared tag) = 8 banks.
            with tc.tile_pool(name="psA", bufs=3, space="PSUM") as psA, \
                 tc.tile_pool(name="psQ", bufs=2, space="PSUM") as psQ, \
                 tc.tile_pool(name="prod", bufs=3) as prod:

                # Stage E (emitted first so it fills early PE gaps):
                # entity embeddings, span mean folded into the selectors.
                for dt in range(6):
                    ee_ps = psQ.tile([128, 2 * NP], F32, tag="qe")
                    sp_sl = spans_sb[:, dt * 128:(dt + 1) * 128]
                    nc.tensor.matmul(out=ee_ps[:, 0:NP], lhsT=sp_sl, rhs=sels_sb[0:64, :])
                    nc.tensor.matmul(out=ee_ps[:, NP:2 * NP], lhsT=sp_sl, rhs=selo_sb[0:64, :])
                    nc.scalar.copy(embsT[:, dt, :], ee_ps[:, 0:NP])
                    nc.scalar.copy(embsT[:, 6 + dt, :], ee_ps[:, NP:2 * NP])

                # Stage A: per-pair attention rows via one-hot matmuls
                # (2 heads stacked per PSUM tile), pair products + head sums.
                qp4 = consts.tile([NP, 4], F32)
                for lq in range(4):
                    pm = prod.tile([128, 6, LQ], BF16, tag="prods")
                    for hp in range(6):
                        a_s = psA.tile([128, LQ], F32, tag="as")
                        a_o = psA.tile([128, LQ], F32, tag="ao")
                        lo = g_sb[0:64, hp, lq * LQ:(lq + 1) * LQ]
                        hi = g_sb[64:128, hp, lq * LQ:(lq + 1) * LQ]
                        nc.tensor.matmul(out=a_s[0:64, :], lhsT=sels_sb[0:64, :], rhs=lo)
                        nc.tensor.matmul(out=a_s[64:128, :], lhsT=sels_sb[64:128, :], rhs=hi)
                        nc.tensor.matmul(out=a_o[0:64, :], lhsT=selo_sb[0:64, :], rhs=lo)
                        nc.tensor.matmul(out=a_o[64:128, :], lhsT=selo_sb[64:128, :], rhs=hi)
                        # 2-input DVE ops may read at most one PSUM operand:
                        # stage the S side through SBUF on the scalar engine
                        as_sb = prod.tile([128, LQ], F32, tag="as_sb")
                        nc.scalar.copy(as_sb, a_s)
                        nc.vector.tensor_mul(pm[:, hp, :], as_sb, a_o)
                    # head-sum tree, merged ops
                    nc.vector.tensor_add(pm[:, 0:3, :], pm[:, 0:3, :], pm[:, 3:6, :])
                    nc.vector.tensor_add(pm[:, 0, :], pm[:, 0, :], pm[:, 1, :])
                    nc.vector.tensor_add(pm[:, 0, :], pm[:, 0, :], pm[:, 2, :])
                    # fold the two stacked head-halves across partitions
                    qt = psQ.tile([NP, LQ], F32, tag="qe")
                    nc.tensor.matmul(out=qt, lhsT=fold2_sb, rhs=pm[:, 0, :])
                    nc.vector.reduce_sum(qp4[:, lq:lq + 1], qt, axis=mybir.AxisListType.X)
                    nc.scalar.copy(q_sb[:, lq * LQ:(lq + 1) * LQ], qt)
                    for k in range(4):
                        ch = lq * 4 + k
                        qT_ps = psQ.tile([128, NP], F32, tag="qe")
                        nc.tensor.transpose(
                            qT_ps, q_sb[:, ch * 128:(ch + 1) * 128], id_sb[0:NP, 0:NP]
                        )
                        nc.scalar.copy(aT_sb[:, ch, :], qT_ps)

                # Stage B: 1/rowsum(q)
                qsum = consts.tile([NP, 1], F32)
                nc.vector.reduce_sum(qsum, qp4, axis=mybir.AxisListType.X)
                rq = consts.tile([NP, 1], F32)
                nc.vector.reciprocal(rq, qsum)

            with tc.tile_pool(name="psC", bufs=1, space="PSUM") as psC, \
                 tc.tile_pool(name="psF", bufs=1, space="PSUM") as psF:
                # Stage D: c_raw[p, d] = qT.T @ seq, accumulated over L-chunks
                c_ps0 = psC.tile([NP, 384], F32, tag="c0")
                c_ps1 = psC.tile([NP, 384], F32, tag="c1")
                for ch in range(16):
                    nc.tensor.matmul(out=c_ps0, lhsT=aT_sb[:, ch, :],
                                     rhs=seq_sb[:, ch, 0:384],
                                     start=(ch == 0), stop=(ch == 15))
                    nc.tensor.matmul(out=c_ps1, lhsT=aT_sb[:, ch, :],
                                     rhs=seq_sb[:, ch, 384:768],
                                     start=(ch == 0), stop=(ch == 15))
                # normalize by 1/rowsum(q) (per-partition scalar), transpose
                # into embsT layout [d, p] (bf16)
                c_sb = consts.tile([NP, D], F32)
                nc.vector.tensor_scalar_mul(c_sb[:, 0:384], c_ps0, rq)
                nc.vector.tensor_scalar_mul(c_sb[:, 384:768], c_ps1, rq)
                for dt in range(6):
                    cT_ps = psC.tile([128, NP], F32, tag="cT", bufs=2)
                    nc.tensor.transpose(
                        cT_ps, c_sb[:, dt * 128:(dt + 1) * 128], id_sb[0:NP, 0:NP]
                    )
                    nc.vector.tensor_copy(embsT[:, 12 + dt, :], cT_ps)
                # Stage F: scores = [rel; nota] @ embs, then transpose + NOTA max
                sc_ps = psF.tile([RN, NP], F32, tag="sc")
                for kc in range(18):
                    nc.tensor.matmul(
                        out=sc_ps,
                        lhsT=rel_sb[:, kc, :],
                        rhs=embsT[:, kc, :],
                        start=(kc == 0),
                        stop=(kc == 17),
                    )
                sc_sb = consts.tile([RN, NP], F32)
                nc.vector.tensor_copy(sc_sb, sc_ps)
                scT_ps = psF.tile([NP, RN], F32, tag="scT")
                nc.tensor.transpose(scT_ps, sc_sb, id_sb[0:RN, 0:RN])
                nota = consts.tile([NP, 1], F32)
                nc.vector.reduce_max(nota, scT_ps[:, R:RN], axis=mybir.AxisListType.X)
                nc.vector.tensor_copy(fin[:, 1:R + 1], scT_ps[:, 0:R])
                nc.vector.tensor_copy(fin[:, 0:1], nota)

            nc.sync.dma_start(out=out[:, :], in_=fin)

    return nc


def kernel(sequence_output, attention, relation_embeddings, nota_embeddings,
           span_starts):
    global LAST_RESULTS
    sequence_output = np.asarray(sequence_output, np.float32)
    attention = np.asarray(attention, np.float32)
    span_starts = np.asarray(span_starts)
    rel_t = np.ascontiguousarray(
        np.concatenate(
            [np.asarray(relation_embeddings, np.float32),
             np.asarray(nota_embeddings, np.float32)], axis=0
        ).T
    )

    in_maps = []
    for c in range(NCORES):
        b, g = divmod(c, 4)
        ents = GROUP_ENTS[g]
        rows = np.concatenate(
            [np.arange(span_starts[b, e], span_starts[b, e] + W) for e in ents]
        )
        att_rows = attention[b][:, rows, :].reshape(H * NEW, L)
        sel_s, sel_o = _sel_matrices(g)
        in_maps.append({
            "att_g": np.ascontiguousarray(att_rows.astype(NP_BF16)),
            "seq": np.ascontiguousarray(sequence_output[b].astype(NP_BF16)),
            "spans": np.ascontiguousarray(sequence_output[b][rows].astype(NP_BF16)),
            "sel_s": np.ascontiguousarray(np.concatenate([sel_s, sel_s], 0).astype(NP_BF16)),
            "sel_o": np.ascontiguousarray(np.concatenate([sel_o, sel_o], 0).astype(NP_BF16)),
            "rel_t": rel_t.astype(NP_BF16),
            "fold2": FOLD2.astype(NP_BF16),
        })

    nc = _build_program()
    nc.finalize()  # Bacc legalization (wait splitting, reg alloc)
    LAST_RESULTS = run_bass_kernel_spmd(nc, in_maps, core_ids=list(range(NCORES)))

    out = np.zeros((B, len(ALL_PAIRS), R + 1), np.float32)
    for c in range(NCORES):
        b, g = divmod(c, 4)
        idxs = GROUP_IDX[g]
        out[b, idxs, :] = LAST_RESULTS.results[c]["out"][: len(idxs)]
    return out
